# revision 18
# baseline (speedup 1.0000x reference)
"""Involution2d (B=8, C=256, H=W=56, K=7, G=16, reduction=4) on 8 TRN2 NeuronCores.

Sharding: spatial over H (7 output rows per core, 3-row halos), batch kept
on-chip.  The involution multiply-reduce runs on the Vector engine with a
partition layout of (group g, batch b) = 16*8 = 128 partitions, so the
per-pixel kernel map broadcasts across the 16 channels of its group via a
free-dim step-0 access pattern (no materialized broadcast), and the 49 tap
shifts are plain free-dim offsets into a zero-padded x tile.

Pipeline per core:
  1. DMA x slab into two layouts (bf16): matmul layout [c, (b,h,w)] and
     involution layout [(g,b), (c', h_pad, w_pad)] (+ a 1-element-shifted
     copy so odd-kw taps stay 4-byte aligned for the DVE 2x perf mode).
  2. PE: z = w_reduce @ x (K=256), ker = w_span @ z (K=64), bias-added and
     cast to bf16 on the Scalar engine during PSUM drain.
  3. DMA rearrange ker [o=(g,k), (b,p)] -> [(g,b), (k,p)] via a DRAM bounce.
  4. DVE: for each of 49 taps: prod = x_shift * ker_bcast (bf16, 2x mode),
     row-of-7 partial sums in bf16, master accumulate in fp32.
  5. DMA out.
"""

import os
import sys

import numpy as np

for _p in ("/opt/trn_rl_repo",):
    if os.path.isdir(_p) and _p not in sys.path:
        sys.path.insert(0, _p)

import concourse.bass as bass
import concourse.bacc as bacc
import concourse.mybir as mybir
from concourse.tile import TileContext
from concourse.bass_utils import run_bass_kernel_spmd

# Problem constants (hardcoded per the task contract).
B, C, H, W = 8, 256, 56, 56
G, K, PAD = 16, 7, 3
CPG = C // G            # 16 channels per group
KK = K * K              # 49 taps
CR = 64                 # reduced channels
NCORES = 8
HS = H // NCORES        # 7 rows per core
HALO = PAD
HP = HS + 2 * HALO      # 13 padded rows
LPAD = 4                # left W-pad (even, so bf16 stays 4B-aligned)
WP = 64                 # padded row width: 4 + 56 + 4
NPIX = HS * W           # 392 pixels per sample slab
NALL = B * NPIX         # 3136 output pixels per slab
NALLP = B * HS * WP     # 3584 matmul moving dim (keeps W-pad cols)
XFLAT = CPG * HP * WP   # 13312 flat x elems per partition

F32 = mybir.dt.float32
BF16 = mybir.dt.bfloat16

MCHUNK = 2 * KK         # 98 ker rows per matmul chunk (2 groups)
NCHUNKS = G // 2        # 8 chunks


def _build(reps=1):
    nc = bacc.Bacc(trn_type="TRN2")

    xs = nc.dram_tensor("xs", [B, C, HP, WP], F32, kind="ExternalInput").ap()
    w1t = nc.dram_tensor("w1t", [C, CR], F32, kind="ExternalInput").ap()
    b1 = nc.dram_tensor("b1", [CR, 1], F32, kind="ExternalInput").ap()
    w2t = nc.dram_tensor("w2t", [CR, G * KK], F32, kind="ExternalInput").ap()
    b2 = nc.dram_tensor("b2", [G * KK, 1], F32, kind="ExternalInput").ap()
    out = nc.dram_tensor("out", [B, C, HS, W], F32, kind="ExternalOutput").ap()
    kscratch = nc.dram_tensor(
        "kscratch", [reps, NCHUNKS, MCHUNK, NALLP], BF16
    ).ap()

    with TileContext(nc) as tc:
        with (
            tc.tile_pool(name="const", bufs=1) as cpool,
            tc.tile_pool(name="xp", bufs=1) as xpool,
            tc.tile_pool(name="work", bufs=1) as wpool,
            tc.tile_pool(name="stage", bufs=2) as spool,
            tc.tile_pool(name="psum", bufs=1, space="PSUM") as ppool,
        ):
            # ---------------- weights / biases ----------------
            lhsT1 = []
            for i in range(2):
                t = cpool.tile([128, CR], BF16, tag=f"w1_{i}", name=f"w1_{i}")
                nc.gpsimd.dma_start(out=t[:, :], in_=w1t[i * 128:(i + 1) * 128, :])
                lhsT1.append(t)
            lhsT2 = []
            b2t = []
            for j in range(NCHUNKS):
                t = cpool.tile([CR, MCHUNK], BF16, tag=f"w2_{j}", name=f"w2_{j}")
                nc.gpsimd.dma_start(out=t[:, :], in_=w2t[:, j * MCHUNK:(j + 1) * MCHUNK])
                lhsT2.append(t)
                tb = cpool.tile([MCHUNK, 1], F32, tag=f"b2_{j}", name=f"b2_{j}")
                nc.sync.dma_start(out=tb[:, :], in_=b2[j * MCHUNK:(j + 1) * MCHUNK, :])
                b2t.append(tb)
            b1t = cpool.tile([CR, 1], F32, tag="b1", name="b1")
            nc.sync.dma_start(out=b1t[:, :], in_=b1[:, :])

            # ---------------- x loads ----------------
            # involution layout, even phase: [(g,b), (c', h_pad, w_pad)]
            x_even = xpool.tile([128, CPG, HP, WP], BF16, tag="xe", name="x_even")
            x_odd = xpool.tile([128, CPG, HP, WP], BF16, tag="xo", name="x_odd")
            x_even_f = x_even.rearrange("p a c d -> p (a c d)")
            x_odd_f = x_odd.rearrange("p a c d -> p (a c d)")
            xs_g = xs.rearrange("b (g c) h w -> g b (c h w)", g=G)
            nc.gpsimd.dma_start(out=x_even_f[:, :], in_=xs_g)
            # odd phase: same data shifted one element so odd-kw taps
            # stay 4-byte aligned in bf16
            nc.gpsimd.dma_start(out=x_odd_f[:, 0:XFLAT - 1], in_=xs_g[:, :, 1:])

            # matmul layout: [c, (b, h, w)] interior pixels only
            xs_int = xs[:, :, HALO:HALO + HS, :].rearrange(
                "b c h w -> c b (h w)"
            )
            xmm = []
            for i in range(2):
                t = xpool.tile([128, NALLP], BF16, tag=f"xmm_{i}", name=f"xmm_{i}")
                nc.gpsimd.dma_start(
                    out=t[:, :], in_=xs_int[i * 128:(i + 1) * 128]
                )
                xmm.append(t)

            # ---------------- kernel generation ----------------
            nsplits = []
            n0 = 0
            while n0 < NALLP:
                nsplits.append((n0, min(NALLP, n0 + 512)))
                n0 += 512

            z_sb = wpool.tile([CR, NALLP], BF16, tag="z", name="z_sb")
            ker_t = wpool.tile([128, KK, HS, WP], BF16, tag="kt", name="ker_t")
            acc = wpool.tile([128, CPG, HS, W], F32, tag="acc", name="acc")
            part = wpool.tile([128, CPG, HS, W], BF16, tag="part", name="part")
            prod = wpool.tile([128, CPG, HS, W], BF16, tag="prod", name="prod")
            out_r = out.rearrange("b (g c) h w -> g b (c h w)", g=G)
            acc_flat = acc.rearrange("p a c d -> p (a c d)")

            for rep in range(reps):
                psum_z = ppool.tile(
                    [CR, NALLP], F32, tag="ps", name=f"psum_z{rep}"
                )
                for i in range(2):
                    for (a, b_) in nsplits:
                        nc.tensor.matmul(
                            out=psum_z[:, a:b_],
                            lhsT=lhsT1[i][:, :],
                            rhs=xmm[i][:, a:b_],
                            start=(i == 0),
                            stop=(i == 1),
                        )
                nc.scalar.add(z_sb[:, :], psum_z[:, :], b1t[:, 0:1])

                for j in range(NCHUNKS):
                    psum_k = ppool.tile(
                        [MCHUNK, NALLP], F32, tag="ps", name=f"psum_k{rep}_{j}"
                    )
                    for (a, b_) in nsplits:
                        nc.tensor.matmul(
                            out=psum_k[:, a:b_],
                            lhsT=lhsT2[j][:, :],
                            rhs=z_sb[:, a:b_],
                            start=True,
                            stop=True,
                        )
                    kst = spool.tile(
                        [MCHUNK, NALLP], BF16, tag="kst", name=f"kst{rep}_{j}"
                    )
                    nc.scalar.add(kst[:, :], psum_k[:, :], b2t[j][:, 0:1])
                    # bounce through DRAM to exchange partition/free dims:
                    # [o=(g2,k), (b,p)] -> [(g2,b), (k,p)]
                    nc.sync.dma_start(out=kscratch[rep, j, :, :], in_=kst[:, :])
                    for gg in range(2):
                        r0 = j * 16 + gg * 8
                        nc.sync.dma_start(
                            out=ker_t[r0:r0 + 8, :, :, :].rearrange(
                                "b k h w -> b k (h w)"
                            ),
                            in_=kscratch[rep, j, gg * KK:(gg + 1) * KK].rearrange(
                                "k (b p) -> b k p", b=B
                            ),
                        )

                # ---------------- involution on DVE ----------------
                with nc.allow_low_precision("involution bf16 row partials"):
                    for kh in range(K):
                        for kw in range(K):
                            kidx = kh * K + kw
                            cb = kw + (LPAD - PAD)  # padded col base, = kw+1
                            if cb % 2 == 0:
                                xin = x_even[:, :, kh:kh + HS, cb:cb + W]
                            else:
                                xin = x_odd[:, :, kh:kh + HS, cb - 1:cb - 1 + W]
                            kin = ker_t[
                                :, kidx:kidx + 1, :, LPAD:LPAD + W
                            ].broadcast_to(
                                (128, CPG, HS, W)
                            )
                            dst = part if kw == 0 else prod
                            nc.vector.tensor_mul(dst[:, :, :, :], xin, kin)
                            if kw > 0:
                                nc.vector.tensor_add(
                                    part[:, :, :, :],
                                    part[:, :, :, :],
                                    prod[:, :, :, :],
                                )
                        if kh == 0:
                            nc.vector.tensor_copy(acc[:, :, :, :], part[:, :, :, :])
                        else:
                            nc.vector.tensor_add(
                                acc[:, :, :, :], acc[:, :, :, :], part[:, :, :, :]
                            )

                # ---------------- store ----------------
                for g in range(G):
                    nc.sync.dma_start(
                        out=out_r[g], in_=acc_flat[g * B:(g + 1) * B, :]
                    )

    return nc


_CACHE = {}


def _get_program(reps=1):
    if reps not in _CACHE:
        nc = _build(reps)
        nc.compile()
        _CACHE[reps] = nc
    return _CACHE[reps]


def _make_inputs(x, w_reduce, b_reduce, w_span, b_span):
    x = np.ascontiguousarray(np.asarray(x, dtype=np.float32))
    w1t = np.ascontiguousarray(np.asarray(w_reduce, np.float32).T)
    b1 = np.ascontiguousarray(np.asarray(b_reduce, np.float32).reshape(-1, 1))
    w2t = np.ascontiguousarray(np.asarray(w_span, np.float32).T)
    b2 = np.ascontiguousarray(np.asarray(b_span, np.float32).reshape(-1, 1))
    in_maps = []
    for i in range(NCORES):
        h0 = i * HS - HALO
        sl = np.zeros((B, C, HP, WP), np.float32)
        s0, s1 = max(0, h0), min(H, h0 + HP)
        sl[:, :, s0 - h0:s1 - h0, LPAD:LPAD + W] = x[:, :, s0:s1, :]
        in_maps.append({"xs": sl, "w1t": w1t, "b1": b1, "w2t": w2t, "b2": b2})
    return in_maps


def kernel_with_results(x, w_reduce, b_reduce, w_span, b_span, trace=False, reps=1):
    in_maps = _make_inputs(x, w_reduce, b_reduce, w_span, b_span)
    nc = _get_program(reps)
    res = run_bass_kernel_spmd(nc, in_maps, list(range(NCORES)), trace=trace)
    full = np.concatenate(
        [res.results[i]["out"] for i in range(NCORES)], axis=2
    ).astype(np.float32)
    return full, res


def kernel(x, w_reduce, b_reduce, w_span, b_span):
    full, _ = kernel_with_results(x, w_reduce, b_reduce, w_span, b_span)
    return full
